# revision 14
# baseline (speedup 1.0000x reference)
"""Trainium2 Bass kernel for CORAL loss (BCE-with-logits over ordinal levels).

Computes mean(BCEWithLogits(logits, levels)), levels[i,k] = 1(targets[i] > k).

Decomposition (exact):
    bce = relu(x) - x*z + f(|x|),   z = 1(t > k),  f(u) = log1p(exp(-u))
so with host-side row sorting by target (the loss is permutation-invariant):
  * sum relu(x) = (sum x + sum |x|) / 2
  * sum x*z: rows sorted by t DESC make {i : t_i > k} a PREFIX per column k.
    The device emits block-column-sums B[cell, k] over cells of 1024 sorted
    rows (PE ones-matmuls); the host adds full cells below each cutoff
    n_k = #(t > k) plus a <=1023-element boundary correction per column
    computed from the same bf16 values the device saw.
  * sum f(|x|) ~= C0*N + C1 * sum sigmoid(-BETA*|x|)  (minimax fit on
    u in [0,12]; sup err 9.8e-3, mean bias ~+3e-3 of the final value,
    far inside the 2e-2 tolerance). sigma comes from ONE ACT pass with a
    fused accumulator.

Per core / chunk [128 part, 4096 free]:
  DVE: |x| via int16 bitcast AND 0x7fff (4x mode); psum evacuations;
       sum|x| on odd chunks via tensor_scalar cache-reduce (min BIG, add).
  ACT: V = sigmoid(-BETA*|x|), accum_out = per-chunk sum V.
  PE : 8 ones-stationary colsum matmuls -> B psum [1, 512] per chunk;
       global sum|x| colsums on even chunks.

Layout: sorted row i' of the core shard sits at partition p = i' % 128,
row-group g = i' // 128; HBM holds [128, 512*64] contiguous per partition.
"""

import os
import sys

import ml_dtypes
import numpy as np

for _p in (
    "/opt/trn_rl_repo",
    os.path.expanduser("~/.axon_site/_ro/trn_rl_repo"),
):
    if os.path.isdir(_p) and _p not in sys.path:
        sys.path.append(_p)

import concourse.bass as bass  # noqa: E402
import concourse.tile as tile  # noqa: E402
from concourse import bacc, mybir  # noqa: E402
from concourse.bass_utils import run_bass_kernel_spmd  # noqa: E402

N_CORES = 8
B, K = 524288, 64
B_SHARD = B // N_CORES  # 65536 rows per core
P = 128
G = B_SHARD // P  # 512 row-groups per core
CHUNK_G = 64  # row-groups per chunk
N_CHUNKS = G // CHUNK_G  # 8
FD = CHUNK_G * K  # 4096 free elements per chunk
CELL_G = 8  # row-groups per B-cell (cell = 1024 rows)
CELLS_PER_CHUNK = CHUNK_G // CELL_G  # 8
CELL_ROWS = CELL_G * P  # 1024
N_CELLS = B // CELL_ROWS  # 512 cells globally

BETA = 1.22
C0 = 0.00915281
C1 = 1.34834565

_nc_cache = None


def _build():
    f32 = mybir.dt.float32
    bf16 = mybir.dt.bfloat16
    i16 = mybir.dt.int16
    nc = bacc.Bacc(
        "TRN2",
        target_bir_lowering=False,
        debug=False,
        enable_asserts=False,
        num_devices=N_CORES,
    )
    x_d = nc.dram_tensor("xs", [P, G * K], bf16, kind="ExternalInput").ap()
    b_d = nc.dram_tensor("Bcol", [1, N_CHUNKS * 512], f32, kind="ExternalOutput").ap()
    su_d = nc.dram_tensor("SU", [1, 512], f32, kind="ExternalOutput").ap()
    accv_d = nc.dram_tensor("accV", [P, N_CHUNKS + 2], f32, kind="ExternalOutput").ap()
    accu_d = nc.dram_tensor("accU", [P, 3], f32, kind="ExternalOutput").ap()

    with tile.TileContext(nc) as tc:
        with (
            tc.tile_pool(name="xp", bufs=N_CHUNKS) as xpool,
            tc.tile_pool(name="up", bufs=7) as upool,
            tc.tile_pool(name="vp", bufs=3) as vpool,
            tc.tile_pool(name="qp", bufs=2) as qpool,
            tc.tile_pool(name="misc", bufs=1) as mpool,
            tc.tile_pool(name="bps", bufs=3, space="PSUM") as bpsum,
            tc.tile_pool(name="sups", bufs=1, space="PSUM") as supsum,
        ):
            h = FD // 2
            xts = {}
            for c in range(N_CHUNKS):
                xt = xpool.tile([P, FD], bf16, tag="x")
                nc.sync.dma_start(xt[:, :h], x_d[:, c * FD : c * FD + h])
                nc.sync.dma_start(xt[:, h:], x_d[:, c * FD + h : (c + 1) * FD])
                xts[c] = xt

            ones_sb = mpool.tile([P, 1], bf16, tag="ones")
            nc.vector.memset(ones_sb[:], 1.0)
            b_sb = mpool.tile([1, N_CHUNKS * 512], f32, tag="bsb")
            su_sb = mpool.tile([1, 512], f32, tag="susb")
            accv = mpool.tile([P, N_CHUNKS + 2], f32, tag="accv")
            accu = mpool.tile([P, 3], f32, tag="accu")

            sup = supsum.tile([1, 512], f32, tag="sup")
            pe_su_chunks = [0, 2, 4, 6, 7]
            cr_chunks = {1: 0, 3: 1, 5: 2}
            cr_pending = []

            # software pipeline: |x| (DVE, 4x via sign-bit AND on int16 view)
            # is issued one chunk ahead of the sigmoid so CR/copy ops never
            # stall the ACT engine. Chunk 0 is split so ACT starts on the
            # first DMA quarter.
            q1 = FD // 4
            uts = {}

            def issue_abs(c):
                ut = upool.tile([P, FD], bf16, tag="u")
                spans = [(0, q1), (q1, h), (h, FD)] if c == 0 else [(0, FD)]
                for lo, hi in spans:
                    nc.vector.tensor_scalar(
                        ut[:, lo:hi].bitcast(i16),
                        xts[c][:, lo:hi].bitcast(i16),
                        0x7FFF,
                        None,
                        mybir.AluOpType.bitwise_and,
                    )
                uts[c] = ut

            issue_abs(0)
            for c in range(N_CHUNKS):
                xt = xts[c]
                ut = uts.pop(c)
                if c + 1 < N_CHUNKS:
                    issue_abs(c + 1)

                vt = vpool.tile([P, FD], bf16, tag="v")
                spans = [(0, q1), (q1, h), (h, FD)] if c == 0 else [(0, FD)]
                for si, (lo, hi) in enumerate(spans):
                    # V = sigmoid(-BETA*|x|); accum -> sum V for this span
                    nc.scalar.activation(
                        vt[:, lo:hi],
                        ut[:, lo:hi],
                        mybir.ActivationFunctionType.Sigmoid,
                        scale=-BETA,
                        accum_out=accv[:, c + si : c + si + 1] if c == 0 else accv[:, c + 2 : c + 3],
                    )

                # B colsums: 8 ones-matmuls accumulating j inside each cell
                x4 = xt[:].rearrange("p (cell j k) -> p cell j k", cell=CELLS_PER_CHUNK, j=CELL_G)
                bp = bpsum.tile([1, 512], f32, tag="bp")
                for j in range(CELL_G):
                    nc.tensor.matmul(
                        bp[:].rearrange("o (cell k) -> o cell k", k=K),
                        ones_sb[:],
                        x4[:, :, j, :],
                        start=(j == 0),
                        stop=(j == CELL_G - 1),
                    )
                nc.vector.tensor_copy(b_sb[:, c * 512 : (c + 1) * 512], bp[:])
                nc.sync.dma_start(b_d[:, c * 512 : (c + 1) * 512], b_sb[:, c * 512 : (c + 1) * 512])

                if c in pe_su_chunks:
                    # global sum|x| via PE colsums (accumulated across chunks)
                    u4 = ut[:].rearrange(
                        "p (cell j k) -> p cell j k", cell=CELLS_PER_CHUNK, j=CELL_G
                    )
                    ci = pe_su_chunks.index(c)
                    for j in range(CELL_G):
                        nc.tensor.matmul(
                            sup[:].rearrange("o (cell k) -> o cell k", k=K),
                            ones_sb[:],
                            u4[:, :, j, :],
                            start=(ci == 0 and j == 0),
                            stop=(ci == len(pe_su_chunks) - 1 and j == CELL_G - 1),
                            skip_group_check=True,
                        )
                else:
                    # defer sum|x| cache-reduce until after every AND has
                    # issued so a 4.4us CR never stalls the ACT feed chain
                    cr_pending.append((c, ut))

            for c, ut in cr_pending:
                qt = qpool.tile([P, FD], bf16, tag="q")
                nc.vector.tensor_scalar(
                    qt[:],
                    ut[:],
                    3.0e38,
                    0.0,
                    mybir.AluOpType.min,
                    mybir.AluOpType.add,
                    accum_out=accu[:, cr_chunks[c] : cr_chunks[c] + 1],
                )
            nc.vector.tensor_copy(su_sb[:], sup[:])
            nc.sync.dma_start(su_d[:], su_sb[:])
            nc.sync.dma_start(accv_d[:], accv[:])
            nc.sync.dma_start(accu_d[:], accu[:])

    nc.compile()
    return nc


def _get_nc():
    global _nc_cache
    if _nc_cache is None:
        _nc_cache = _build()
    return _nc_cache


def run(logits, targets, **spmd_kwargs):
    """Host prep (sort by target desc), 8-core SPMD run, host assembly."""
    nc = _get_nc()
    logits = np.asarray(logits)
    targets = np.asarray(targets)
    assert logits.shape == (B, K), logits.shape
    assert targets.shape == (B,), targets.shape

    order = np.argsort(-targets.astype(np.int64), kind="stable")
    t_sorted = targets[order]
    xs = logits[order].astype(ml_dtypes.bfloat16)  # [B, K] sorted desc by t

    # per-core tile layout: sorted row i' = g*128 + p  ->  [P, G*K]
    lg = np.ascontiguousarray(
        xs.reshape(N_CORES, G, P, K).transpose(0, 2, 1, 3).reshape(N_CORES, P, G * K)
    )

    in_maps = [{"xs": lg[c]} for c in range(N_CORES)]
    res = run_bass_kernel_spmd(nc, in_maps, core_ids=list(range(N_CORES)), **spmd_kwargs)

    xs64 = None  # lazily materialized boundary rows only

    # gather device outputs
    Bcol = np.zeros((N_CELLS, K), dtype=np.float64)  # global cells x K
    sum_absx = 0.0
    sum_v = 0.0
    for ci, r in enumerate(res.results):
        bc = r["Bcol"].astype(np.float64).reshape(N_CHUNKS * CELLS_PER_CHUNK, K)
        Bcol[ci * 64 : (ci + 1) * 64] = bc
        sum_absx += r["SU"].astype(np.float64).sum() + r["accU"].astype(np.float64).sum()
        sum_v += r["accV"].astype(np.float64).sum()

    sum_x = Bcol.sum()

    # sum x*z: per column k, prefix of n_k = #(t > k) sorted rows
    ks = np.arange(K)
    n_k = np.count_nonzero(t_sorted[:, None] > ks[None, :], axis=0)  # [K]
    full_cells = n_k // CELL_ROWS
    sum_xz = 0.0
    for k in range(K):
        m = full_cells[k]
        sum_xz += Bcol[:m, k].sum()
        lo, hi = m * CELL_ROWS, n_k[k]
        if hi > lo:
            sum_xz += xs[lo:hi, k].astype(np.float64).sum()

    n_total = float(B) * K
    sum_relu = 0.5 * (sum_x + sum_absx)
    sum_f = C0 * n_total + C1 * sum_v
    total = sum_relu - sum_xz + sum_f
    mean = total / n_total
    return np.float32(mean), res


def kernel(logits, targets):
    out, _ = run(logits, targets)
    return out


# revision 15
# speedup vs baseline: 1.1440x; 1.1440x over previous
"""Trainium2 Bass kernel for CORAL loss (BCE-with-logits over ordinal levels).

Computes mean(BCEWithLogits(logits, levels)), levels[i,k] = 1(targets[i] > k).

Decomposition (exact):
    bce = relu(x) - x*z + f(|x|),   z = 1(t > k),  f(u) = log1p(exp(-u))
so with host-side row sorting by target (the loss is permutation-invariant):
  * sum relu(x) = (sum x + sum |x|) / 2
  * sum x*z: rows sorted by t DESC make {i : t_i > k} a PREFIX per column k.
    The device emits block-column-sums B[cell, k] over cells of 1024 sorted
    rows (PE ones-matmuls); the host adds full cells below each cutoff
    n_k = #(t > k) plus a <=1023-element boundary correction per column
    computed from the same bf16 values the device saw.
  * sum f(|x|) ~= C0*N + C1 * sum sigmoid(-BETA*|x|)  (minimax fit on
    u in [0,12]; sup err 9.8e-3, mean bias ~+3e-3 of the final value,
    far inside the 2e-2 tolerance). sigma comes from ONE ACT pass with a
    fused accumulator.

Per core / chunk [128 part, 4096 free]:
  DVE: |x| via int16 bitcast AND 0x7fff (4x mode); psum evacuations;
       sum|x| on odd chunks via tensor_scalar cache-reduce (min BIG, add).
  ACT: V = sigmoid(-BETA*|x|), accum_out = per-chunk sum V.
  PE : 8 ones-stationary colsum matmuls -> B psum [1, 512] per chunk;
       global sum|x| colsums on even chunks.

Layout: sorted row i' of the core shard sits at partition p = i' % 128,
row-group g = i' // 128; HBM holds [128, 512*64] contiguous per partition.
"""

import os
import sys

import ml_dtypes
import numpy as np

for _p in (
    "/opt/trn_rl_repo",
    os.path.expanduser("~/.axon_site/_ro/trn_rl_repo"),
):
    if os.path.isdir(_p) and _p not in sys.path:
        sys.path.append(_p)

import concourse.bass as bass  # noqa: E402
import concourse.tile as tile  # noqa: E402
from concourse import bacc, mybir  # noqa: E402
from concourse.bass_utils import run_bass_kernel_spmd  # noqa: E402

N_CORES = 8
B, K = 524288, 64
B_SHARD = B // N_CORES  # 65536 rows per core
P = 128
G = B_SHARD // P  # 512 row-groups per core
CHUNK_G = 64  # row-groups per chunk
N_CHUNKS = G // CHUNK_G  # 8
FD = CHUNK_G * K  # 4096 free elements per chunk
CELL_G = 8  # row-groups per B-cell (cell = 1024 rows)
CELLS_PER_CHUNK = CHUNK_G // CELL_G  # 8
CELL_ROWS = CELL_G * P  # 1024
N_CELLS = B // CELL_ROWS  # 512 cells globally

BETA = 1.22
C0 = 0.00915281
C1 = 1.34834565

_nc_cache = None


def _build():
    f32 = mybir.dt.float32
    bf16 = mybir.dt.bfloat16
    i16 = mybir.dt.int16
    nc = bacc.Bacc(
        "TRN2",
        target_bir_lowering=False,
        debug=False,
        enable_asserts=False,
        num_devices=N_CORES,
    )
    x_d = nc.dram_tensor("xs", [P, G * K], bf16, kind="ExternalInput").ap()
    b_d = nc.dram_tensor("Bcol", [1, N_CHUNKS * 512], f32, kind="ExternalOutput").ap()
    su_d = nc.dram_tensor("SU", [1, 512], f32, kind="ExternalOutput").ap()
    accv_d = nc.dram_tensor("accV", [P, N_CHUNKS + 2], f32, kind="ExternalOutput").ap()
    accu_d = nc.dram_tensor("accU", [P, 3], f32, kind="ExternalOutput").ap()

    with tile.TileContext(nc) as tc:
        with (
            tc.tile_pool(name="xp", bufs=N_CHUNKS) as xpool,
            tc.tile_pool(name="up", bufs=7) as upool,
            tc.tile_pool(name="vp", bufs=2) as vpool,
            tc.tile_pool(name="qp", bufs=2) as qpool,
            tc.tile_pool(name="misc", bufs=1) as mpool,
            tc.tile_pool(name="bps", bufs=3, space="PSUM") as bpsum,
            tc.tile_pool(name="sups", bufs=1, space="PSUM") as supsum,
        ):
            h = FD // 2
            xts = {}
            for c in range(N_CHUNKS):
                xt = xpool.tile([P, FD], bf16, tag="x")
                nc.sync.dma_start(xt[:, :h], x_d[:, c * FD : c * FD + h])
                nc.sync.dma_start(xt[:, h:], x_d[:, c * FD + h : (c + 1) * FD])
                xts[c] = xt

            ones_sb = mpool.tile([P, 1], bf16, tag="ones")
            nc.vector.memset(ones_sb[:], 1.0)
            b_sb = mpool.tile([1, N_CHUNKS * 512], f32, tag="bsb")
            su_sb = mpool.tile([1, 512], f32, tag="susb")
            accv = mpool.tile([P, N_CHUNKS + 2], f32, tag="accv")
            accu = mpool.tile([P, 3], f32, tag="accu")

            sup = supsum.tile([1, 512], f32, tag="sup")
            pe_su_chunks = [0, 2, 4, 6, 7]
            cr_chunks = {1: 0, 3: 1, 5: 2}
            cr_pending = []

            # software pipeline: |x| (DVE, 4x via sign-bit AND on int16 view)
            # is issued one chunk ahead of the sigmoid so CR/copy ops never
            # stall the ACT engine. Chunk 0 is split so ACT starts on the
            # first DMA quarter.
            q1 = FD // 4
            uts = {}

            def issue_abs(c):
                ut = upool.tile([P, FD], bf16, tag="u")
                spans = [(0, q1), (q1, h), (h, FD)] if c == 0 else [(0, FD)]
                for lo, hi in spans:
                    nc.vector.tensor_scalar(
                        ut[:, lo:hi].bitcast(i16),
                        xts[c][:, lo:hi].bitcast(i16),
                        0x7FFF,
                        None,
                        mybir.AluOpType.bitwise_and,
                    )
                uts[c] = ut

            issue_abs(0)
            for c in range(N_CHUNKS):
                xt = xts[c]
                ut = uts.pop(c)
                if c + 1 < N_CHUNKS:
                    issue_abs(c + 1)

                vt = vpool.tile([P, FD], bf16, tag="v")
                spans = [(0, q1), (q1, h), (h, FD)] if c == 0 else [(0, FD)]
                for si, (lo, hi) in enumerate(spans):
                    # V = sigmoid(-BETA*|x|); accum -> sum V for this span
                    nc.scalar.activation(
                        vt[:, lo:hi],
                        ut[:, lo:hi],
                        mybir.ActivationFunctionType.Sigmoid,
                        scale=-BETA,
                        accum_out=accv[:, c + si : c + si + 1] if c == 0 else accv[:, c + 2 : c + 3],
                    )

                # B colsums: 8 ones-matmuls accumulating j inside each cell
                x4 = xt[:].rearrange("p (cell j k) -> p cell j k", cell=CELLS_PER_CHUNK, j=CELL_G)
                bp = bpsum.tile([1, 512], f32, tag="bp")
                for j in range(CELL_G):
                    nc.tensor.matmul(
                        bp[:].rearrange("o (cell k) -> o cell k", k=K),
                        ones_sb[:],
                        x4[:, :, j, :],
                        start=(j == 0),
                        stop=(j == CELL_G - 1),
                    )
                nc.vector.tensor_copy(b_sb[:, c * 512 : (c + 1) * 512], bp[:])
                nc.sync.dma_start(b_d[:, c * 512 : (c + 1) * 512], b_sb[:, c * 512 : (c + 1) * 512])

                if c in pe_su_chunks:
                    # global sum|x| via PE colsums (accumulated across chunks)
                    u4 = ut[:].rearrange(
                        "p (cell j k) -> p cell j k", cell=CELLS_PER_CHUNK, j=CELL_G
                    )
                    ci = pe_su_chunks.index(c)
                    for j in range(CELL_G):
                        nc.tensor.matmul(
                            sup[:].rearrange("o (cell k) -> o cell k", k=K),
                            ones_sb[:],
                            u4[:, :, j, :],
                            start=(ci == 0 and j == 0),
                            stop=(ci == len(pe_su_chunks) - 1 and j == CELL_G - 1),
                            skip_group_check=True,
                        )
                else:
                    # defer sum|x| cache-reduce until after every AND has
                    # issued so a 4.4us CR never stalls the ACT feed chain
                    cr_pending.append((c, ut))

            for c, ut in cr_pending:
                qt = qpool.tile([P, FD], bf16, tag="q")
                nc.vector.tensor_scalar(
                    qt[:],
                    ut[:],
                    3.0e38,
                    0.0,
                    mybir.AluOpType.min,
                    mybir.AluOpType.add,
                    accum_out=accu[:, cr_chunks[c] : cr_chunks[c] + 1],
                )
            nc.vector.tensor_copy(su_sb[:], sup[:])
            nc.sync.dma_start(su_d[:], su_sb[:])
            nc.sync.dma_start(accv_d[:], accv[:])
            nc.sync.dma_start(accu_d[:], accu[:])

    nc.compile()
    return nc


def _get_nc():
    global _nc_cache
    if _nc_cache is None:
        _nc_cache = _build()
    return _nc_cache


def run(logits, targets, **spmd_kwargs):
    """Host prep (sort by target desc), 8-core SPMD run, host assembly."""
    nc = _get_nc()
    logits = np.asarray(logits)
    targets = np.asarray(targets)
    assert logits.shape == (B, K), logits.shape
    assert targets.shape == (B,), targets.shape

    order = np.argsort(-targets.astype(np.int64), kind="stable")
    t_sorted = targets[order]
    xs = logits[order].astype(ml_dtypes.bfloat16)  # [B, K] sorted desc by t

    # per-core tile layout: sorted row i' = g*128 + p  ->  [P, G*K]
    lg = np.ascontiguousarray(
        xs.reshape(N_CORES, G, P, K).transpose(0, 2, 1, 3).reshape(N_CORES, P, G * K)
    )

    in_maps = [{"xs": lg[c]} for c in range(N_CORES)]
    res = run_bass_kernel_spmd(nc, in_maps, core_ids=list(range(N_CORES)), **spmd_kwargs)

    xs64 = None  # lazily materialized boundary rows only

    # gather device outputs
    Bcol = np.zeros((N_CELLS, K), dtype=np.float64)  # global cells x K
    sum_absx = 0.0
    sum_v = 0.0
    for ci, r in enumerate(res.results):
        bc = r["Bcol"].astype(np.float64).reshape(N_CHUNKS * CELLS_PER_CHUNK, K)
        Bcol[ci * 64 : (ci + 1) * 64] = bc
        sum_absx += r["SU"].astype(np.float64).sum() + r["accU"].astype(np.float64).sum()
        sum_v += r["accV"].astype(np.float64).sum()

    sum_x = Bcol.sum()

    # sum x*z: per column k, prefix of n_k = #(t > k) sorted rows
    ks = np.arange(K)
    n_k = np.count_nonzero(t_sorted[:, None] > ks[None, :], axis=0)  # [K]
    full_cells = n_k // CELL_ROWS
    sum_xz = 0.0
    for k in range(K):
        m = full_cells[k]
        sum_xz += Bcol[:m, k].sum()
        lo, hi = m * CELL_ROWS, n_k[k]
        if hi > lo:
            sum_xz += xs[lo:hi, k].astype(np.float64).sum()

    n_total = float(B) * K
    sum_relu = 0.5 * (sum_x + sum_absx)
    sum_f = C0 * n_total + C1 * sum_v
    total = sum_relu - sum_xz + sum_f
    mean = total / n_total
    return np.float32(mean), res


def kernel(logits, targets):
    out, _ = run(logits, targets)
    return out


# revision 17
# speedup vs baseline: 1.1649x; 1.0183x over previous
"""Trainium2 Bass kernel for CORAL loss (BCE-with-logits over ordinal levels).

Computes mean(BCEWithLogits(logits, levels)), levels[i,k] = 1(targets[i] > k).

Decomposition (exact):
    bce = relu(x) - x*z + f(|x|),   z = 1(t > k),  f(u) = log1p(exp(-u))
so with host-side row sorting by target (the loss is permutation-invariant):
  * sum relu(x) = (sum x + sum |x|) / 2
  * sum x*z: rows sorted by t DESC make {i : t_i > k} a PREFIX per column k.
    The device emits block-column-sums B[cell, k] over cells of 1024 sorted
    rows (PE ones-matmuls); the host adds full cells below each cutoff
    n_k = #(t > k) plus a <=1023-element boundary correction per column
    computed from the same bf16 values the device saw.
  * sum f(|x|) ~= C0*N + C1 * sum sigmoid(-BETA*|x|)  (minimax fit on
    u in [0,12]; sup err 9.8e-3, mean bias ~+3e-3 of the final value,
    far inside the 2e-2 tolerance). sigma comes from ONE ACT pass with a
    fused accumulator.

Per core / chunk [128 part, 4096 free]:
  DVE: |x| via int16 bitcast AND 0x7fff (4x mode); psum evacuations;
       sum|x| on odd chunks via tensor_scalar cache-reduce (min BIG, add).
  ACT: V = sigmoid(-BETA*|x|), accum_out = per-chunk sum V.
  PE : 8 ones-stationary colsum matmuls -> B psum [1, 512] per chunk;
       global sum|x| colsums on even chunks.

Layout: sorted row i' of the core shard sits at partition p = i' % 128,
row-group g = i' // 128; HBM holds [128, 512*64] contiguous per partition.
"""

import os
import sys

import ml_dtypes
import numpy as np

for _p in (
    "/opt/trn_rl_repo",
    os.path.expanduser("~/.axon_site/_ro/trn_rl_repo"),
):
    if os.path.isdir(_p) and _p not in sys.path:
        sys.path.append(_p)

import concourse.bass as bass  # noqa: E402
import concourse.tile as tile  # noqa: E402
from concourse import bacc, mybir  # noqa: E402
from concourse.bass_utils import run_bass_kernel_spmd  # noqa: E402

N_CORES = 8
B, K = 524288, 64
B_SHARD = B // N_CORES  # 65536 rows per core
P = 128
G = B_SHARD // P  # 512 row-groups per core
CHUNK_G = 64  # row-groups per chunk
N_CHUNKS = G // CHUNK_G  # 8
FD = CHUNK_G * K  # 4096 free elements per chunk
CELL_G = 8  # row-groups per B-cell (cell = 1024 rows)
CELLS_PER_CHUNK = CHUNK_G // CELL_G  # 8
CELL_ROWS = CELL_G * P  # 1024
N_CELLS = B // CELL_ROWS  # 512 cells globally

BETA = 1.22
C0 = 0.00915281
C1 = 1.34834565

_nc_cache = None


def _build():
    f32 = mybir.dt.float32
    bf16 = mybir.dt.bfloat16
    i16 = mybir.dt.int16
    nc = bacc.Bacc(
        "TRN2",
        target_bir_lowering=False,
        debug=False,
        enable_asserts=False,
        num_devices=N_CORES,
    )
    x_d = nc.dram_tensor("xs", [P, G * K], bf16, kind="ExternalInput").ap()
    b_d = nc.dram_tensor("Bcol", [1, N_CHUNKS * 512], f32, kind="ExternalOutput").ap()
    su_d = nc.dram_tensor("SU", [1, 512], f32, kind="ExternalOutput").ap()
    accv_d = nc.dram_tensor("accV", [P, 7], f32, kind="ExternalOutput").ap()
    accu_d = nc.dram_tensor("accU", [P, 3], f32, kind="ExternalOutput").ap()

    with tile.TileContext(nc) as tc:
        with (
            tc.tile_pool(name="xp", bufs=N_CHUNKS) as xpool,
            tc.tile_pool(name="up", bufs=4) as upool,
            tc.tile_pool(name="vp", bufs=2) as vpool,
            tc.tile_pool(name="qp", bufs=2) as qpool,
            tc.tile_pool(name="misc", bufs=1) as mpool,
            tc.tile_pool(name="bps", bufs=7, space="PSUM") as bpsum,
            tc.tile_pool(name="sups", bufs=1, space="PSUM") as supsum,
        ):
            h = FD // 2
            xts = {}
            for c in range(N_CHUNKS):
                xt = xpool.tile([P, FD], bf16, tag="x")
                nc.sync.dma_start(xt[:, :h], x_d[:, c * FD : c * FD + h])
                nc.sync.dma_start(xt[:, h:], x_d[:, c * FD + h : (c + 1) * FD])
                xts[c] = xt

            ones_sb = mpool.tile([P, 1], bf16, tag="ones")
            nc.vector.memset(ones_sb[:], 1.0)
            b_sb = mpool.tile([1, N_CHUNKS * 512], f32, tag="bsb")
            su_sb = mpool.tile([1, 512], f32, tag="susb")
            accv = mpool.tile([P, 7], f32, tag="accv")
            accu = mpool.tile([P, 3], f32, tag="accu")

            sup = supsum.tile([1, 512], f32, tag="sup")
            pe_su_chunks = [0, 2, 4, 6, 7]
            cr_chunks = {1: 0, 3: 1, 5: 2}
            cr_pending = []
            b_pending = []

            # software pipeline: |x| (DVE, 4x via sign-bit AND on int16 view)
            # is issued one chunk ahead of the sigmoid so CR/copy ops never
            # stall the ACT engine. Chunk 0 is split so ACT starts on the
            # first DMA quarter.
            q1 = FD // 4
            upairs = {}
            uview = {}

            def issue_abs(c):
                pi = c // 2
                if pi not in upairs:
                    up_t = upool.tile([P, 2 * FD], bf16, tag="u")
                    upairs[pi] = up_t
                off = (c % 2) * FD
                ut = upairs[pi]
                spans = [(0, q1), (q1, h), (h, FD)] if c == 0 else [(0, FD)]
                for lo, hi in spans:
                    nc.vector.tensor_scalar(
                        ut[:, off + lo : off + hi].bitcast(i16),
                        xts[c][:, lo:hi].bitcast(i16),
                        0x7FFF,
                        None,
                        mybir.AluOpType.bitwise_and,
                    )
                uview[c] = ut[:, off : off + FD]

            issue_abs(0)
            acc_col = 0
            for c in range(N_CHUNKS):
                xt = xts[c]
                if c + 1 < N_CHUNKS:
                    issue_abs(c + 1)

                if c == 0:
                    vt = vpool.tile([P, FD], bf16, tag="v")
                    for lo, hi in [(0, q1), (q1, h), (h, FD)]:
                        nc.scalar.activation(
                            vt[:, lo:hi],
                            uview[0][:, lo:hi],
                            mybir.ActivationFunctionType.Sigmoid,
                            scale=-BETA,
                            accum_out=accv[:, acc_col : acc_col + 1],
                        )
                        acc_col += 1
                elif c == 1:
                    vt = vpool.tile([P, FD], bf16, tag="v")
                    nc.scalar.activation(
                        vt[:],
                        uview[1],
                        mybir.ActivationFunctionType.Sigmoid,
                        scale=-BETA,
                        accum_out=accv[:, acc_col : acc_col + 1],
                    )
                    acc_col += 1
                elif c % 2 == 1:
                    vt = vpool.tile([P, 2 * FD], bf16, tag="v")
                    nc.scalar.activation(
                        vt[:],
                        upairs[c // 2][:],
                        mybir.ActivationFunctionType.Sigmoid,
                        scale=-BETA,
                        accum_out=accv[:, acc_col : acc_col + 1],
                    )
                    acc_col += 1

                # B colsums: 8 ones-matmuls accumulating j inside each cell
                x4 = xt[:].rearrange("p (cell j k) -> p cell j k", cell=CELLS_PER_CHUNK, j=CELL_G)
                bp = bpsum.tile([1, 512], f32, tag="bp")
                for j in range(CELL_G):
                    nc.tensor.matmul(
                        bp[:].rearrange("o (cell k) -> o cell k", k=K),
                        ones_sb[:],
                        x4[:, :, j, :],
                        start=(j == 0),
                        stop=(j == CELL_G - 1),
                    )
                if c >= 6:
                    nc.vector.tensor_copy(b_sb[:, c * 512 : (c + 1) * 512], bp[:])
                    nc.sync.dma_start(b_d[:, c * 512 : (c + 1) * 512], b_sb[:, c * 512 : (c + 1) * 512])
                else:
                    b_pending.append((c, bp))

                if c in pe_su_chunks:
                    # global sum|x| via PE colsums (accumulated across chunks)
                    u4 = uview[c].rearrange(
                        "p (cell j k) -> p cell j k", cell=CELLS_PER_CHUNK, j=CELL_G
                    )
                    ci = pe_su_chunks.index(c)
                    for j in range(CELL_G):
                        nc.tensor.matmul(
                            sup[:].rearrange("o (cell k) -> o cell k", k=K),
                            ones_sb[:],
                            u4[:, :, j, :],
                            start=(ci == 0 and j == 0),
                            stop=(ci == len(pe_su_chunks) - 1 and j == CELL_G - 1),
                            skip_group_check=True,
                        )
                else:
                    # defer sum|x| cache-reduce until after every AND has
                    # issued so a 4.4us CR never stalls the ACT feed chain
                    cr_pending.append((c, uview[c]))

            for c, bp in b_pending:
                nc.vector.tensor_copy(b_sb[:, c * 512 : (c + 1) * 512], bp[:])
                nc.sync.dma_start(b_d[:, c * 512 : (c + 1) * 512], b_sb[:, c * 512 : (c + 1) * 512])
            for c, uv in cr_pending:
                qt = qpool.tile([P, FD], bf16, tag="q")
                nc.vector.tensor_scalar(
                    qt[:],
                    uv,
                    3.0e38,
                    0.0,
                    mybir.AluOpType.min,
                    mybir.AluOpType.add,
                    accum_out=accu[:, cr_chunks[c] : cr_chunks[c] + 1],
                )
            nc.vector.tensor_copy(su_sb[:], sup[:])
            nc.sync.dma_start(su_d[:], su_sb[:])
            nc.sync.dma_start(accv_d[:], accv[:])
            nc.sync.dma_start(accu_d[:], accu[:])

    nc.compile()
    return nc


def _get_nc():
    global _nc_cache
    if _nc_cache is None:
        _nc_cache = _build()
    return _nc_cache


def run(logits, targets, **spmd_kwargs):
    """Host prep (sort by target desc), 8-core SPMD run, host assembly."""
    nc = _get_nc()
    logits = np.asarray(logits)
    targets = np.asarray(targets)
    assert logits.shape == (B, K), logits.shape
    assert targets.shape == (B,), targets.shape

    order = np.argsort(-targets.astype(np.int64), kind="stable")
    t_sorted = targets[order]
    xs = logits[order].astype(ml_dtypes.bfloat16)  # [B, K] sorted desc by t

    # per-core tile layout: sorted row i' = g*128 + p  ->  [P, G*K]
    lg = np.ascontiguousarray(
        xs.reshape(N_CORES, G, P, K).transpose(0, 2, 1, 3).reshape(N_CORES, P, G * K)
    )

    in_maps = [{"xs": lg[c]} for c in range(N_CORES)]
    res = run_bass_kernel_spmd(nc, in_maps, core_ids=list(range(N_CORES)), **spmd_kwargs)

    xs64 = None  # lazily materialized boundary rows only

    # gather device outputs
    Bcol = np.zeros((N_CELLS, K), dtype=np.float64)  # global cells x K
    sum_absx = 0.0
    sum_v = 0.0
    for ci, r in enumerate(res.results):
        bc = r["Bcol"].astype(np.float64).reshape(N_CHUNKS * CELLS_PER_CHUNK, K)
        Bcol[ci * 64 : (ci + 1) * 64] = bc
        sum_absx += r["SU"].astype(np.float64).sum() + r["accU"].astype(np.float64).sum()
        sum_v += r["accV"].astype(np.float64).sum()

    sum_x = Bcol.sum()

    # sum x*z: per column k, prefix of n_k = #(t > k) sorted rows
    ks = np.arange(K)
    n_k = np.count_nonzero(t_sorted[:, None] > ks[None, :], axis=0)  # [K]
    full_cells = n_k // CELL_ROWS
    sum_xz = 0.0
    for k in range(K):
        m = full_cells[k]
        sum_xz += Bcol[:m, k].sum()
        lo, hi = m * CELL_ROWS, n_k[k]
        if hi > lo:
            sum_xz += xs[lo:hi, k].astype(np.float64).sum()

    n_total = float(B) * K
    sum_relu = 0.5 * (sum_x + sum_absx)
    sum_f = C0 * n_total + C1 * sum_v
    total = sum_relu - sum_xz + sum_f
    mean = total / n_total
    return np.float32(mean), res


def kernel(logits, targets):
    out, _ = run(logits, targets)
    return out
